# revision 9
# baseline (speedup 1.0000x reference)
"""Enformer dot-product self-attention with central-mask relative position
bias, on 8 Trainium2 NeuronCores (one head per core, SPMD).

Math per head h (S=2048, D=64, N=64):
    basis[i,j,:] = f(d=i-j)  — indicator features, zero for |d| > 1024
    logits = (q @ k^T + (q @ w) @ basis^T + u @ k^T + (v @ w) @ basis^T) / 8
    out    = softmax(logits) @ value

Device formulation per core:
  - qT_aug [65,S]: rows 0..63 = (q/8)^T, row 64 = ones.
  - k_augT [65,S]: rows 0..63 = k^T (loaded), row 64 = (u/8) @ k^T (computed).
    qk+uk logits tile = qT_aug_tile^T @ k_augT  (PE, fp32r).
  - Band term: T[i,c'] = qT_aug[:,i] . w2r[:,c'] where c' = j-i+1024 in
    [0,2048], w2r[65, 2176] host-built from w (rows 0..63) and v@w/8 (row 64),
    columns >= 2049 are zero padding.  T is written to DRAM [S rows, pitch
    2176]; the diagonal (rel-shift) read back uses a flat access pattern with
    row step 2175 so each partition p lands j-aligned; out-of-band elements
    read the zero padding of the previous row.
  - softmax: exp without max subtraction (logit range is small), ScalarE
    activation with accum_out giving per-row denominators for free.
  - attn@v: P cast to bf16, transposed [128,128]-blockwise with the DMA xbar
    transpose, then out^T[d,i] accumulated in PSUM over key blocks with
    lhsT = v (natural layout).  Final PE transpose + per-row 1/denom scale.
"""

import numpy as np
import ml_dtypes

import concourse.bass as bass
import concourse.bacc as bacc
import concourse.mybir as mybir
import concourse.tile as tile
from concourse.bass_utils import run_bass_kernel_spmd
from concourse.masks import make_identity

S = 2048
D = 64
NB = 64          # pos-emb dim (basis features)
H = 8
HALF = NB // 2   # 32
BAND = 1024      # max |d| with nonzero features
Q = S + 128      # G row pitch (2049 band cols + 127 zero pad)
F32 = mybir.dt.float32
F32R = mybir.dt.float32r
BF16 = mybir.dt.bfloat16

_NC_CACHE = {}


def _basis_feature_matrix():
    """Rr[c', n] for c' in [0, Q): features of distance d = 1024 - c'.
    Matches reference._relative_basis numerics (float32)."""
    pow_rate = np.float32(np.exp(np.log((S + 1) / 2) / HALF))
    widths = np.power(pow_rate, np.arange(1, HALF + 1, dtype=np.float32),
                      dtype=np.float32)  # [32]
    d = (np.float32(BAND) - np.arange(Q, dtype=np.float32))[:, None]  # [Q,1]
    unsigned = (np.abs(d) <= widths[None, :]).astype(np.float32)      # [Q,32]
    signed = np.sign(d) * unsigned
    return np.concatenate([unsigned, signed], axis=-1)  # [Q, 64]


def _build_nc():
    if "nc" in _NC_CACHE:
        return _NC_CACHE["nc"]

    nc = bacc.Bacc("TRN2", target_bir_lowering=False, debug=False,
                   num_devices=H)
    d_qT = nc.dram_tensor("qT_aug", [65, S], F32R, kind="ExternalInput")
    d_kT = nc.dram_tensor("kT", [D, S], F32R, kind="ExternalInput")
    d_u8 = nc.dram_tensor("u8", [D, 1], F32R, kind="ExternalInput")
    d_v = nc.dram_tensor("vb", [S, D], BF16, kind="ExternalInput")
    d_w2r = nc.dram_tensor("w2r", [65, Q], F32R, kind="ExternalInput")
    d_out = nc.dram_tensor("out", [S, D], F32, kind="ExternalOutput")

    NT = S // 128  # 16 i/j tiles
    d_Gs = [nc.dram_tensor(f"gband{t}", [128 * Q], F32, kind="Internal")
            for t in range(NT)]

    with tile.TileContext(nc) as tc:
        with tc.tile_pool(name="pers", bufs=1) as pers:
            sb_w2r = pers.tile([65, Q], F32R)
            nc.sync.dma_start(out=sb_w2r[:], in_=d_w2r[:])
            sb_qT = pers.tile([65, S], F32R)
            nc.sync.dma_start(out=sb_qT[:], in_=d_qT[:])
            sb_kaug = pers.tile([65, S], F32R)
            nc.sync.dma_start(out=sb_kaug[0:D, :], in_=d_kT[:])
            sb_v = pers.tile([128, NT, D], BF16)
            for t in range(NT):
                nc.sync.dma_start(out=sb_v[:, t, :],
                                  in_=d_v[t * 128:(t + 1) * 128, :])
            sb_id = pers.tile([128, 128], F32)
            make_identity(nc, sb_id[:])
            sb_idb = pers.tile([128, 128], BF16)
            make_identity(nc, sb_idb[:])
            sb_P = pers.tile([128, NT, S], BF16)      # exp(logits), i-tiled
            sb_oT = pers.tile([D, S], F32)            # out^T
            sb_den = pers.tile([128, NT], F32)        # softmax denominators
            sb_u8 = pers.tile([D, 1], F32R)
            nc.sync.dma_start(out=sb_u8[:], in_=d_u8[:])

            # ---- uk = (u/8) @ k^T  ->  k_augT row 64 ----
            with tc.tile_pool(name="ps_uk", bufs=2, space="PSUM") as ps_uk:
                for c in range(4):
                    pk = ps_uk.tile([1, 512], F32)
                    nc.tensor.matmul(
                        pk[:],
                        lhsT=sb_u8[:],
                        rhs=sb_kaug[0:D, c * 512:(c + 1) * 512],
                        start=True, stop=True)
                    nc.scalar.copy(out=sb_kaug[64:65, c * 512:(c + 1) * 512],
                                   in_=pk[:])

            # ---- Phases A+C interleaved per i-tile ----
            # A(t): band tile -> PSUM -> SBUF -> DRAM G_t.
            # C(t): qk matmuls -> PSUM; band read back (rel-shift diagonal
            # AP on G_t); DVE add; ScalarE exp (+denominator accum).
            with tc.tile_pool(name="gsb", bufs=3) as gsb, \
                 tc.tile_pool(name="bandsb", bufs=3) as bsb, \
                 tc.tile_pool(name="ps_g", bufs=3, space="PSUM") as psg, \
                 tc.tile_pool(name="ps_qk", bufs=1, space="PSUM") as psqk:
                for t in range(NT):
                    i0 = t * 128
                    # A(t): band matrix rows for this tile
                    gt = gsb.tile([128, Q], F32)
                    for c in range(5):
                        w = 512 if c < 4 else Q - 2048  # 4x512 + 128
                        pg = psg.tile([128, 512], F32, tag="gchunk")
                        nc.tensor.matmul(
                            pg[:, 0:w],
                            lhsT=sb_qT[:, i0:i0 + 128],
                            rhs=sb_w2r[:, c * 512:c * 512 + w],
                            start=True, stop=True)
                        if c % 2 == 0:
                            nc.scalar.copy(out=gt[:, c * 512:c * 512 + w],
                                           in_=pg[:, 0:w])
                        else:
                            nc.vector.tensor_copy(gt[:, c * 512:c * 512 + w],
                                                  pg[:, 0:w])
                    wr = bass.AP(tensor=d_Gs[t], offset=0,
                                 ap=[[Q, 128], [1, Q]])
                    nc.gpsimd.dma_start(out=wr, in_=gt[:])

                    # C(t): qk+uk logits, band add, exp
                    pq = psqk.tile([128, S], F32)
                    for c in range(4):
                        nc.tensor.matmul(
                            pq[:, c * 512:(c + 1) * 512],
                            lhsT=sb_qT[:, i0:i0 + 128],
                            rhs=sb_kaug[:, c * 512:(c + 1) * 512],
                            start=True, stop=True)
                    jlo = max(0, i0 - BAND)
                    jhi = min(S, i0 + 128 + BAND)
                    wdt = jhi - jlo
                    bt = bsb.tile([128, Q], F32)
                    rd = bass.AP(tensor=d_Gs[t], offset=(jlo - i0 + BAND),
                                 ap=[[Q - 1, 128], [1, wdt]])
                    nc.gpsimd.dma_start(out=bt[:, 0:wdt], in_=rd)
                    nc.vector.tensor_add(pq[:, jlo:jhi], pq[:, jlo:jhi],
                                         bt[:, 0:wdt])
                    nc.scalar.activation(out=sb_P[:, t, :], in_=pq[:],
                                         func=mybir.ActivationFunctionType.Exp,
                                         accum_out=sb_den[:, t:t + 1])

            # ---- Phase D+E: transpose P blocks (PE), accumulate out^T ----
            CW = 512  # i-chunk width
            NC_ = S // CW
            with tc.tile_pool(name="ptsb", bufs=4) as ptsb, \
                 tc.tile_pool(name="ps_t", bufs=2, space="PSUM") as pst, \
                 tc.tile_pool(name="ps_av", bufs=2, space="PSUM") as psav, \
                 tc.tile_pool(name="ps_f", bufs=2, space="PSUM") as psf, \
                 tc.tile_pool(name="fsb", bufs=3) as fsb:
                for c in range(NC_):  # i-chunks
                    po = psav.tile([D, CW], F32)
                    for jb in range(NT):
                        ptp = pst.tile([128, CW], BF16)
                        for s in range(CW // 128):
                            t = (CW // 128) * c + s
                            nc.tensor.transpose(
                                ptp[:, s * 128:(s + 1) * 128],
                                sb_P[:, t, jb * 128:(jb + 1) * 128],
                                sb_idb[:])
                        pt = ptsb.tile([128, CW], BF16)
                        if jb % 2 == 0:
                            nc.scalar.copy(out=pt[:], in_=ptp[:])
                        else:
                            nc.vector.tensor_copy(pt[:], ptp[:])
                        nc.tensor.matmul(po[:], lhsT=sb_v[:, jb, :],
                                         rhs=pt[:],
                                         start=(jb == 0), stop=(jb == NT - 1))
                    nc.vector.tensor_copy(sb_oT[:, c * CW:(c + 1) * CW],
                                          po[:])

                # ---- Phase F: transpose out^T, normalize, store ----
                for t in range(NT):
                    pf = psf.tile([128, D], F32)
                    nc.tensor.transpose(pf[:],
                                        sb_oT[:, t * 128:(t + 1) * 128],
                                        sb_id[0:D, 0:D])
                    rc = fsb.tile([128, 1], F32, tag="rc")
                    nc.vector.reciprocal(rc[:], sb_den[:, t:t + 1])
                    ot = fsb.tile([128, D], F32, tag="ot")
                    nc.vector.tensor_scalar_mul(ot[:], pf[:], rc[:])
                    nc.sync.dma_start(out=d_out[t * 128:(t + 1) * 128, :],
                                      in_=ot[:])

    nc.finalize()
    _NC_CACHE["nc"] = nc
    return nc


def _host_prep(query, key, value, u, v, w):
    """Build the 8 per-core input maps from the full inputs."""
    q = np.asarray(query, np.float32)[0]   # [S,H,D]
    k = np.asarray(key, np.float32)[0]
    val = np.asarray(value, np.float32)[0]
    u = np.asarray(u, np.float32)
    v = np.asarray(v, np.float32)
    w = np.asarray(w, np.float32)
    Rr = _basis_feature_matrix()           # [Q, 64]

    ones = np.ones((1, S), np.float32)
    in_maps = []
    for h in range(H):
        qT8 = np.ascontiguousarray(q[:, h, :].T) / np.float32(8.0)  # [64,S]
        qT_aug = np.concatenate([qT8, ones], axis=0)                # [65,S]
        kT = np.ascontiguousarray(k[:, h, :].T)                     # [64,S]
        u8 = (u[h] / np.float32(8.0)).reshape(D, 1)
        vb = val[:, h, :].astype(ml_dtypes.bfloat16)                # [S,64]
        w2r_qr = w[h] @ Rr.T                                        # [64,Q]
        vw8 = (v[h] @ w[h]) / np.float32(8.0)                       # [64]
        w2r_vr = (vw8 @ Rr.T).reshape(1, Q)                         # [1,Q]
        w2r = np.concatenate([w2r_qr, w2r_vr], axis=0).astype(np.float32)
        in_maps.append({
            "qT_aug": np.ascontiguousarray(qT_aug),
            "kT": kT,
            "u8": np.ascontiguousarray(u8),
            "vb": np.ascontiguousarray(vb),
            "w2r": np.ascontiguousarray(w2r),
        })
    return in_maps


def kernel(query, key, value, u, v, w, _trace=False):
    nc = _build_nc()
    in_maps = _host_prep(query, key, value, u, v, w)
    res = run_bass_kernel_spmd(nc, in_maps, core_ids=list(range(H)),
                               trace=_trace)
    outs = np.stack([res.results[h]["out"] for h in range(H)])  # [H,S,D]
    full = np.transpose(outs, (1, 0, 2))[None]                  # [1,S,H,D]
    out = np.ascontiguousarray(full.astype(np.float32))
    if _trace:
        return out, res
    return out


if __name__ == "__main__":
    rng = np.random.default_rng(0)
    ins = {
        "query": rng.standard_normal((1, S, H, D), np.float32),
        "key": rng.standard_normal((1, S, H, D), np.float32),
        "value": rng.standard_normal((1, S, H, D), np.float32),
        "u": rng.standard_normal((H, D), np.float32),
        "v": rng.standard_normal((H, D), np.float32),
        "w": rng.standard_normal((H, D, NB), np.float32),
    }
    out = kernel(**ins)
    print("out shape:", out.shape, "finite:", np.isfinite(out).all())
